# revision 56
# baseline (speedup 1.0000x reference)
"""Binarized 3x3 conv (BConv2d) on 8 TRN2 NeuronCores — Winograd F(2,3) along H.

Problem: x (32, 32, 256, 256) f32, weight (32, 32, 3, 3) f32.
  out = conv2d(x, sign(weight), padding='same') / sqrt(32*9)

Strategy:
  - Data-parallel over batch: core i gets images 4i..4i+3 (no collectives).
  - Per core, pack 4 images x 32 input channels onto the 128 SBUF
    partitions; weights are block-diagonal (per-image) as in the direct
    kernel.
  - 1D Winograd F(2,3) over the kh (row) axis: for each output row pair
    (2r, 2r+1), four transformed input rows V0..V3 (cheap row add/subs on
    DVE/Pool) feed 4 PSUM accumulation chains m0..m3, each a 3-tap kw
    convolution (matmuls over K=128).  out[2r] = m0+m1+m2,
    out[2r+1] = m1-m2-m3 (signs/scale folded into the transformed
    weights).  This cuts PE column count 1.5x vs the 9-tap direct form:
    12 matmuls of N=512 per 2 row-pairs instead of 9 of N=512 per pair.
  - Engine split so nothing passes the PE: DVE computes V0/V1 + the
    output combine adds; Pool (GpSimd) computes V2/V3; Act drains
    m0,m1,m2 from PSUM to fp16; DVE's last add reads m3 straight from
    PSUM.
  - fp16 inputs/outputs (host-side fp16 marshalling), fp32 PSUM
    accumulate; transformed weights carry the 1/sqrt(288) scale and the
    /2 Winograd factors.
"""

import numpy as np
import ml_dtypes

import concourse.mybir as mybir
import concourse.tile as tile
from concourse import bacc
from concourse import bass_utils

N_CORES = 8
N_IMG = 4          # images per core
C_IN = 32
C_OUT = 32
K = 3
H = 256
W = 256
DIV = float(np.sqrt(C_IN * K * K))
N_WT = 12          # 4 winograd positions x 3 kw taps


def build_conv_kernel(
    nimg=N_IMG,
    cin=C_IN,
    cout=C_OUT,
    h=H,
    w=W,
    chunk_rows=16,  # input DMA granularity (rows)
    vblock=4,       # r-pairs per V-transform block (8 input rows)
    div=DIV,
    repeats=1,      # execute the whole body N times (for delta-timing)
    warmup_mms=15,  # zero-weight matmuls (N=256) to cover the input wait
):
    """Build the per-core Bass graph.  Returns nc (compiled Bacc)."""
    P = nimg * cin
    assert P <= 128
    M = nimg * cout
    assert M <= 128
    hp, wp = h + 2, w + 2
    nr = h // 2                     # number of output row pairs
    assert nr % vblock == 0

    nc = bacc.Bacc(
        "TRN2", target_bir_lowering=False, debug=False, num_devices=N_CORES
    )
    x_dram = nc.dram_tensor("x", [P, h, w], mybir.dt.float16, kind="ExternalInput")
    w_dram = nc.dram_tensor(
        "w12", [P, N_WT, M], mybir.dt.float16, kind="ExternalInput"
    )
    out_dram = nc.dram_tensor(
        "out", [M, h, w], mybir.dt.float16, kind="ExternalOutput"
    )
    # V tile row pitch: w data cols + 2 pad cols, padded to a multiple of
    # 16 elems (32B SBUF lines) so no V row shares a line with its
    # neighbor.  Rows of every other tile are 512B (w fp16) and therefore
    # line-aligned already.  Misaligned rows let two engines write the
    # same SBUF line concurrently, which corrupts data on real HW.
    wv = w + 16

    with tile.TileContext(nc) as tc:
        with (
            tc.tile_pool(name="persist", bufs=1) as perpool,
            tc.tile_pool(name="stage", bufs=2) as spool,
            tc.tile_pool(name="ostage", bufs=3) as opool,
            tc.tile_pool(name="psum", bufs=8, space="PSUM") as ppool,
        ):
            # x rows with 1-row top/bottom zero padding; no column padding
            # (every row is a single 512B DMA write)
            xpad = perpool.tile([P, hp, w], mybir.dt.float16, name="xpad")
            wsb = perpool.tile([P, N_WT, M], mybir.dt.float16, name="wsb")
            wz = perpool.tile([P, M], mybir.dt.float16, name="wz")
            wmr = perpool.tile([P, w], mybir.dt.float16, name="wmr")
            # manual triple-buffer of V tiles: two blocks of lookahead so
            # the slower Pool V-ops never gate the PE (persistent so the
            # pad-column memsets below cover every buffer once, up front)
            vts = [
                perpool.tile([P, vblock, 4, wv], mybir.dt.float16, name=f"vt{i}")
                for i in range(3)
            ]
            # warmup operands on Pool (idle at t=0) so DVE's first op is
            # the first V transform; both tiles have line-aligned rows so
            # the cross-engine read is safe
            nc.gpsimd.memset(wz[:], 0.0)
            nc.gpsimd.memset(wmr[:], 0.0)

            def emit_input_rows(r0, nrows):
                nc.sync.dma_start(
                    out=xpad[:, r0 + 1 : r0 + nrows + 1, :],
                    in_=x_dram[:, r0 : r0 + nrows, :],
                )

            # first input piece (just enough for the first 2-pair V block),
            # then weights in first-use order (chains run a=1,2,3,0 so taps
            # 3..11 are needed before 0..2)
            first_piece = 5
            emit_input_rows(0, first_piece)
            nc.sync.dma_start(out=wsb[:, 3:, :], in_=w_dram[:, 3:, :])
            nc.sync.dma_start(out=wsb[:, 0:3, :], in_=w_dram[:, 0:3, :])
            # top/bottom zero rows (full 512B rows, DVE: same engine as the
            # first V-block ops that read them)
            nc.vector.memset(xpad[:, 0, :], 0.0)
            nc.vector.memset(xpad[:, hp - 1, :], 0.0)
            # V pad columns (cols 0 and w+1 of each (r, a) row) are zero
            # forever; set them once on Pool while nothing else runs
            for vt_ in vts:
                nc.gpsimd.memset(vt_[:, :, :, 0], 0.0)
                nc.gpsimd.memset(vt_[:, :, :, w + 1], 0.0)

            def emit_body(first):
                next_row = [first_piece if first else 0]

                def load_until(row_needed):
                    while next_row[0] < min(row_needed, h):
                        r0 = next_row[0]
                        # small second piece so the second V block is early
                        nrows = 4 if r0 < 10 else chunk_rows
                        nrows = min(nrows, h - r0)
                        emit_input_rows(r0, nrows)
                        next_row[0] += nrows

                # PE warm-up while the first input chunks are in flight.
                if warmup_mms:
                    wpt = ppool.tile(
                        [M, 2, w], mybir.dt.float32, name="wpt", tag="pt",
                    )
                    for _ in range(warmup_mms):
                        nc.tensor.matmul(
                            wpt[:, 0, 0:w], wz[:], wmr[:],
                            start=True, stop=True,
                        )

                def emit_vblock(bi, p0, n):
                    """V transform for r-pairs [p0, p0+n) into buffer bi%3.

                    V0 = xpad[2r]   - xpad[2r+2]          (DVE)
                    V1 = xpad[2r+1] + xpad[2r+2]          (DVE)
                    V2 = xpad[2r+2] - xpad[2r+1]          (Pool)
                    V3 = xpad[2r+1] - xpad[2r+3]          (Pool)
                    (writes cols 1..w of each V row; cols 0/w+1 are pad)
                    """
                    vt = vts[bi % 3]
                    s = 2 * p0  # xpad row of first d0

                    def rows(off):
                        stop = min(s + off + 2 * n, hp)
                        return xpad[:, s + off : stop : 2, :]

                    # engine assignment: first blocks all-DVE for fast start
                    # (Pool's slow per-op rate would gate the early groups).
                    # V1 first: the a=1 chain is the first matmul consumer.
                    nc.vector.tensor_add(vt[:, 0:n, 1, 1 : w + 1], rows(1), rows(2))
                    nc.vector.tensor_sub(vt[:, 0:n, 0, 1 : w + 1], rows(0), rows(2))
                    if bi <= 1:
                        nc.vector.tensor_sub(vt[:, 0:n, 2, 1 : w + 1],
                                             rows(2), rows(1))
                        nc.vector.tensor_sub(vt[:, 0:n, 3, 1 : w + 1],
                                             rows(1), rows(3))
                    else:
                        # Pool is slow per-op: emit in 2-pair halves so the
                        # first consumer group isn't gated on the whole block
                        for q0 in range(0, n, 2):
                            q1 = min(q0 + 2, n)
                            sq = s + 2 * q0

                            def qrows(off):
                                stop = min(sq + off + 2 * (q1 - q0), hp)
                                return xpad[:, sq + off : stop : 2, :]

                            nc.gpsimd.tensor_sub(
                                vt[:, q0:q1, 2, 1 : w + 1], qrows(2), qrows(1))
                            nc.gpsimd.tensor_sub(
                                vt[:, q0:q1, 3, 1 : w + 1], qrows(1), qrows(3))
                    return vt

                # groups of 2 r-pairs; final group split in two 1-pair
                # groups so the post-matmul drain tail is short
                plan = [(r0, 2) for r0 in range(0, nr - 2, 2)]
                plan += [(nr - 2, 1), (nr - 1, 1)]

                # V blocks: two small first blocks so the PE starts early
                vplan = [(0, 2), (2, 2)]
                p = 4
                while p < nr:
                    vplan.append((p, vblock))
                    p += vblock

                def emit_next_vblock():
                    p0, n = vplan[len(vtiles)]
                    # input rows for this vblock + one chunk of lookahead
                    load_until(min(2 * (p0 + n) + 2 + chunk_rows, h))
                    vtiles.append((p0, emit_vblock(len(vtiles), p0, n)))

                # bootstrap all three buffers; thereafter emit block cur+2
                # at the first group of block cur (all consumers of block
                # cur-1 are then emitted, so the 3-deep buffer WAR is safe)
                vtiles = []   # (p0, vt) per emitted block
                emit_next_vblock()
                emit_next_vblock()
                emit_next_vblock()
                # pre-issue the next input chunks before any output DMAs
                # exist to head-of-line-block them on the DMA sequencer
                load_until(2 * vblock * 2 + 2 + 2 * chunk_rows)
                cur = 0
                for r0, npairs in plan:
                    if cur + 1 < len(vtiles) and vtiles[cur + 1][0] <= r0:
                        cur += 1
                        if len(vtiles) < len(vplan):
                            emit_next_vblock()
                    vbase, vt = vtiles[cur]
                    j0 = r0 - vbase

                    # 4 PSUM chains (m0..m3), each 3 kw taps, N=512 (2 r's)
                    pts = [
                        ppool.tile([M, npairs, w], mybir.dt.float32,
                                   name="pt", tag="pt", padded_shape=[M, 2, w])
                        for _ in range(4)
                    ]
                    # a=0 last so the drain->combine chain overlaps the
                    # a=3 matmuls; for the very last group a=3 goes last
                    # instead, so the tail ends on the single short
                    # ot-odd op (uu is ready before the a=3 chain ends)
                    last = r0 == nr - 1
                    for a in ((1, 2, 0, 3) if last else (1, 2, 3, 0)):
                        for kw in range(3):
                            nc.tensor.matmul(
                                pts[a][:],
                                wsb[:, a * 3 + kw, :],
                                vt[:, j0 : j0 + npairs, a, kw : kw + w],
                                start=(kw == 0),
                                stop=(kw == 2),
                            )

                    ot = opool.tile([M, 2 * npairs, w], mybir.dt.float16,
                                    name="ot", tag="ot", padded_shape=[M, 4, w])
                    tt = spool.tile([M, npairs, w], mybir.dt.float16,
                                    name="tt", tag="tt", padded_shape=[M, 2, w])
                    uu = spool.tile([M, npairs, w], mybir.dt.float16,
                                    name="uu", tag="uu", padded_shape=[M, 2, w])
                    # drain m0..m2 to fp16 (Act), combine (DVE)
                    ad = [
                        spool.tile([M, npairs, w], mybir.dt.float16,
                                   name=f"a{a}", tag=f"a{a}",
                                   padded_shape=[M, 2, w])
                        for a in range(3)
                    ]
                    nc.scalar.copy(ad[1][:], pts[1][:])
                    nc.scalar.copy(ad[2][:], pts[2][:])
                    nc.vector.tensor_sub(uu[:], ad[1][:], ad[2][:])
                    if last:
                        nc.scalar.copy(ad[0][:], pts[0][:])
                        nc.vector.tensor_add(tt[:], ad[0][:], ad[1][:])
                        nc.vector.tensor_add(ot[:, 0::2, :], tt[:], ad[2][:])
                        nc.sync.dma_start(
                            out=out_dram[:, 2 * r0 : 2 * r0 + 1, :],
                            in_=ot[:, 0:1, :],
                        )
                        nc.vector.tensor_sub(ot[:, 1::2, :], uu[:], pts[3][:])
                        nc.sync.dma_start(
                            out=out_dram[:, 2 * r0 + 1 : 2 * r0 + 2, :],
                            in_=ot[:, 1:2, :],
                        )
                    else:
                        nc.vector.tensor_sub(ot[:, 1::2, :], uu[:], pts[3][:])
                        nc.scalar.copy(ad[0][:], pts[0][:])
                        nc.vector.tensor_add(tt[:], ad[0][:], ad[1][:])
                        nc.vector.tensor_add(ot[:, 0::2, :], tt[:], ad[2][:])
                        nc.sync.dma_start(
                            out=out_dram[:, 2 * r0 : 2 * r0 + 2 * npairs, :],
                            in_=ot[:],
                        )
                load_until(h)

            for _rep in range(repeats):
                emit_body(first=(_rep == 0))

    nc.compile()
    return nc


def make_weight_tensor(weight, nimg=N_IMG, cin=C_IN, cout=C_OUT):
    """Binarize + F(2,3)-transform + block-diagonalize.

    [cout,cin,3,3] f32 -> [nimg*cin, 12, nimg*cout] fp16 where index
    t = a*3+kw holds u_a[kw]/DIV:
      u0 = w[0], u1 = (w[0]+w[1]+w[2])/2, u2 = (w[0]-w[1]+w[2])/2, u3 = w[2]
    """
    wbin = np.where(weight > 0, 1.0, -1.0).astype(np.float32)  # [co, ci, kh, kw]
    u = np.empty((4, cout, cin, 3), dtype=np.float32)
    u[0] = wbin[:, :, 0, :]
    u[1] = 0.5 * (wbin[:, :, 0, :] + wbin[:, :, 1, :] + wbin[:, :, 2, :])
    u[2] = 0.5 * (wbin[:, :, 0, :] - wbin[:, :, 1, :] + wbin[:, :, 2, :])
    u[3] = wbin[:, :, 2, :]
    u /= DIV
    # [a, co, ci, kw] -> [ci, a*3+kw, co]
    wt = u.transpose(2, 0, 3, 1).reshape(cin, N_WT, cout)
    w12 = np.zeros((nimg * cin, N_WT, nimg * cout), dtype=np.float16)
    for i in range(nimg):
        w12[i * cin : (i + 1) * cin, :, i * cout : (i + 1) * cout] = wt
    return w12


def kernel(x, weight, trace=False, repeats=1, _nc_cache={}):
    """Full-input entry point: x (32,32,256,256) f32, weight (32,32,3,3) f32."""
    x = np.asarray(x, dtype=np.float32)
    x = np.ascontiguousarray(x.astype(np.float16))
    weight = np.asarray(weight, dtype=np.float32)
    n_batch = x.shape[0]
    per_core = n_batch // N_CORES

    if repeats not in _nc_cache:
        _nc_cache[repeats] = build_conv_kernel(repeats=repeats)
    nc = _nc_cache[repeats]

    w12 = make_weight_tensor(weight)
    P = N_IMG * C_IN
    in_maps = [
        {
            "x": x[i * per_core : (i + 1) * per_core].reshape(P, H, W),
            "w12": w12,
        }
        for i in range(N_CORES)
    ]
    try:
        res = bass_utils.run_bass_kernel_spmd(
            nc, in_maps, core_ids=list(range(N_CORES)), trace=trace
        )
    except ModuleNotFoundError:
        res = bass_utils.run_bass_kernel_spmd(
            nc, in_maps, core_ids=list(range(N_CORES)), trace=False
        )
    out = np.concatenate(
        [r["out"].astype(np.float32).reshape(per_core, C_OUT, H, W)
         for r in res.results],
        axis=0,
    )
    if trace:
        kernel.last_results = res
    return out


# revision 57
# speedup vs baseline: 1.1691x; 1.1691x over previous
"""fp8 DoubleRow direct-conv BConv2d on 8 TRN2 NeuronCores.

out = conv2d(x, sign(weight), 'same') / sqrt(288), x (32,32,256,256) f32.

- Data-parallel over batch (4 images x 32 cin = 128 partitions/core).
- Host splits x into fp8_e4m3 planes: hi = fp8(x), lo = fp8(x - hi);
  conv is linear so conv(x) ~ conv(hi) + conv(lo) (~1e-3 rel err).
- The planes are row-interleaved in SBUF ([P, hp, 2, 320]) so a
  DoubleRow matmul's two K-tiles are the (hi, lo) rows at stride 320
  (16-bit ISA AP step limit is +-32767).  Each 3x3 tap x output row is
  ONE DoubleRow matmul: K=256 at 0.5 PE cycles/col -> 18 matmuls of
  N=256 per 2-row strip = 0.96us, vs 1.28us for fp16 F(2,3) Winograd.
- No input transforms: DVE/Pool idle; Act drains PSUM with the 1/17
  scale.  Row pitch 320B = 10 SBUF lines, data at col 32: pad columns
  (31 / 288) share no 32B line with the DMA-written data.
"""

import numpy as np
import ml_dtypes

import concourse.mybir as mybir
import concourse.tile as tile
from concourse import bacc
from concourse import bass_utils

N_CORES = 8
N_IMG = 4
C_IN = 32
C_OUT = 32
H = 256
W = 256
DIV = float(np.sqrt(C_IN * 9))
XPITCH = 320
XCOL = 32


def build_conv_kernel(
    nimg=N_IMG, cin=C_IN, cout=C_OUT, h=H, w=W,
    chunk_rows=16, div=DIV, repeats=1, warmup_mms=16,
):
    P = nimg * cin
    M = nimg * cout
    hp = h + 2
    n_taps = 9

    nc = bacc.Bacc(
        "TRN2", target_bir_lowering=False, debug=False, num_devices=N_CORES
    )
    x_dram = nc.dram_tensor(
        "xq", [P, h, 2, w], mybir.dt.float8e4, kind="ExternalInput"
    )
    w_dram = nc.dram_tensor(
        "w9d", [P, n_taps, 2, M], mybir.dt.float8e4, kind="ExternalInput"
    )
    out_dram = nc.dram_tensor(
        "out", [M, h, w], mybir.dt.float16, kind="ExternalOutput"
    )

    with tile.TileContext(nc) as tc:
        with (
            tc.tile_pool(name="persist", bufs=1) as perpool,
            tc.tile_pool(name="ostage", bufs=4) as opool,
            tc.tile_pool(name="psum", bufs=8, space="PSUM") as ppool,
        ):
            xq = perpool.tile([P, hp, 2, XPITCH], mybir.dt.float8e4, name="xq")
            wsb = perpool.tile([P, n_taps, 2, M], mybir.dt.float8e4, name="wsb")
            wz = perpool.tile([P, 2, M], mybir.dt.float8e4, name="wz")
            wmr = perpool.tile([P, 2, 2 * w], mybir.dt.float8e4, name="wmr")
            nc.gpsimd.memset(wz[:], 0.0)
            nc.gpsimd.memset(wmr[:], 0.0)

            def emit_input_rows(r0, nrows):
                for pl in range(2):
                    nc.sync.dma_start(
                        out=xq[:, r0 + 1 : r0 + nrows + 1, pl, XCOL : XCOL + w],
                        in_=x_dram[:, r0 : r0 + nrows, pl, :],
                    )

            first_piece = 4
            emit_input_rows(0, first_piece)
            nc.sync.dma_start(out=wsb[:], in_=w_dram[:])
            nc.vector.memset(xq[:, 0, :, :], 0.0)
            nc.vector.memset(xq[:, hp - 1, :, :], 0.0)
            nc.vector.memset(xq[:, :, :, XCOL - 1], 0.0)
            nc.vector.memset(xq[:, :, :, XCOL + w], 0.0)

            def emit_body(first):
                next_row = [first_piece if first else 0]

                def load_until(row_needed):
                    while next_row[0] < min(row_needed, h):
                        r0 = next_row[0]
                        nrows = min(4 if r0 < 12 else chunk_rows, h - r0)
                        emit_input_rows(r0, nrows)
                        next_row[0] += nrows

                if warmup_mms:
                    wpt = ppool.tile(
                        [M, 512], mybir.dt.float32, name="wpt", tag="pt",
                        padded_shape=[M, 512],
                    )
                    for _ in range(warmup_mms):
                        nc.tensor.matmul(
                            wpt[:], wz[:], wmr[:], start=True, stop=True,
                            perf_mode=mybir.MatmulPerfMode.DoubleRow,
                        )

                n_strips = h // 2
                ot = None
                for s in range(n_strips):
                    load_until(min(2 * s + 4 + chunk_rows, h))
                    pts = ppool.tile(
                        [M, 2, w], mybir.dt.float32, name="pt", tag="pt",
                    )
                    # complete each row's chain before starting the next:
                    # interleaved DoubleRow accumulation chains on one
                    # PSUM bank corrupt the first chain on real HW
                    for j in range(2):
                        for t in range(n_taps):
                            dy, dx = t // 3, t % 3
                            c = XCOL - 1 + dx
                            nc.tensor.matmul(
                                pts[:, j, :],
                                wsb[:, t, :, :],
                                xq[:, 2 * s + j + dy, :, c : c + w],
                                start=(t == 0),
                                stop=(t == n_taps - 1),
                                perf_mode=mybir.MatmulPerfMode.DoubleRow,
                            )
                    solo = s >= n_strips - 2
                    if s % 2 == 0 or solo:
                        ot = opool.tile(
                            [M, 2 if solo else 4, w], mybir.dt.float16,
                            name="ot", tag="ot", padded_shape=[M, 4, w],
                        )
                    half = 0 if (s % 2 == 0 or solo) else 1
                    nc.scalar.mul(ot[:, 2 * half : 2 * half + 2, :],
                                  pts[:], 1.0 / div)
                    if half == 1 or solo:
                        r0 = 2 * s - (0 if solo else 2)
                        nc.sync.dma_start(
                            out=out_dram[:, r0 : r0 + ot.shape[1], :],
                            in_=ot[:],
                        )
                load_until(h)

            for _rep in range(repeats):
                emit_body(first=(_rep == 0))

    nc.compile()
    return nc


def make_weight_tensor(weight, nimg=N_IMG, cin=C_IN, cout=C_OUT):
    n_taps = 9
    wbin = np.where(weight > 0, 1.0, -1.0).astype(np.float32)
    wt = wbin.reshape(cout, cin, n_taps).transpose(1, 2, 0)
    w9 = np.zeros((nimg * cin, n_taps, nimg * cout), dtype=np.float32)
    for i in range(nimg):
        w9[i * cin : (i + 1) * cin, :, i * cout : (i + 1) * cout] = wt
    return np.repeat(w9[:, :, None, :], 2, axis=2).astype(ml_dtypes.float8_e4m3fn)


def split_fp8(x):
    hi = x.astype(ml_dtypes.float8_e4m3fn)
    lo = (x - hi.astype(np.float32)).astype(ml_dtypes.float8_e4m3fn)
    return hi, lo


def kernel(x, weight, trace=False, repeats=1, _nc_cache={}):
    x = np.asarray(x, dtype=np.float32)
    weight = np.asarray(weight, dtype=np.float32)
    per_core = x.shape[0] // N_CORES
    if repeats not in _nc_cache:
        _nc_cache[repeats] = build_conv_kernel(repeats=repeats)
    nc = _nc_cache[repeats]
    w9d = make_weight_tensor(weight)
    P = N_IMG * C_IN
    hi, lo = split_fp8(x)
    # [cores, P, H, 2, W]: planes row-interleaved
    xq = np.ascontiguousarray(np.stack(
        [hi.reshape(N_CORES, P, H, W), lo.reshape(N_CORES, P, H, W)], axis=3
    ))
    in_maps = [{"xq": xq[i], "w9d": w9d} for i in range(N_CORES)]
    try:
        res = bass_utils.run_bass_kernel_spmd(
            nc, in_maps, core_ids=list(range(N_CORES)), trace=trace
        )
    except ModuleNotFoundError:
        res = bass_utils.run_bass_kernel_spmd(
            nc, in_maps, core_ids=list(range(N_CORES)), trace=False
        )
    out = np.concatenate(
        [r["out"].astype(np.float32).reshape(per_core, C_OUT, H, W)
         for r in res.results], axis=0,
    )
    if trace:
        kernel.last_results = res
    return out


# revision 59
# speedup vs baseline: 1.3158x; 1.1255x over previous
"""fp8 DoubleRow direct-conv BConv2d on 8 TRN2 NeuronCores.

out = conv2d(x, sign(weight), 'same') / sqrt(288), x (32,32,256,256) f32.

- Data-parallel over batch (4 images x 32 cin = 128 partitions/core).
- Host splits x into fp8_e4m3 planes: hi = fp8(x), lo = fp8(x - hi);
  conv is linear so conv(x) ~ conv(hi) + conv(lo) (~1e-3 rel err).
- The planes are row-interleaved in SBUF ([P, hp, 2, 320]) so a
  DoubleRow matmul's two K-tiles are the (hi, lo) rows at stride 320
  (16-bit ISA AP step limit is +-32767).  Each 3x3 tap x output row is
  ONE DoubleRow matmul: K=256 at 0.5 PE cycles/col -> 18 matmuls of
  N=256 per 2-row strip = 0.96us, vs 1.28us for fp16 F(2,3) Winograd.
- No input transforms: DVE/Pool idle; Act drains PSUM with the 1/17
  scale.  Row pitch 320B = 10 SBUF lines, data at col 32: pad columns
  (31 / 288) share no 32B line with the DMA-written data.
"""

import numpy as np
import ml_dtypes

import concourse.mybir as mybir
import concourse.tile as tile
from concourse import bacc
from concourse import bass_utils

N_CORES = 8
N_IMG = 4
C_IN = 32
C_OUT = 32
H = 256
W = 256
DIV = float(np.sqrt(C_IN * 9))
XPITCH = 320
XCOL = 32


def build_conv_kernel(
    nimg=N_IMG, cin=C_IN, cout=C_OUT, h=H, w=W,
    chunk_rows=16, div=DIV, repeats=1, warmup_mms=16,
):
    P = nimg * cin
    M = nimg * cout
    hp = h + 2
    n_taps = 9

    nc = bacc.Bacc(
        "TRN2", target_bir_lowering=False, debug=False, num_devices=N_CORES
    )
    x_dram = nc.dram_tensor(
        "xq", [P, h, 2, w], mybir.dt.float8e4, kind="ExternalInput"
    )
    w_dram = nc.dram_tensor(
        "w9d", [P, n_taps, 2, M], mybir.dt.float8e4, kind="ExternalInput"
    )
    out_dram = nc.dram_tensor(
        "out", [M, h, w], mybir.dt.float16, kind="ExternalOutput"
    )

    with tile.TileContext(nc) as tc:
        with (
            tc.tile_pool(name="persist", bufs=1) as perpool,
            tc.tile_pool(name="ostage", bufs=4) as opool,
            tc.tile_pool(name="psum", bufs=8, space="PSUM") as ppool,
        ):
            # hi|lo packed per row: exactly 512B line-aligned rows so the
            # input DMA moves >=512B runs (1x rate, no sub-512B penalty).
            # No pad columns: the dx edge taps run ragged (the dx=1 tap
            # starts each PSUM chain full-width, dx=0/2 accumulate into
            # shifted 255-col sub-ranges whose edge contribution is zero
            # by 'same' padding anyway).
            xq = perpool.tile([P, hp, 2, w], mybir.dt.float8e4, name="xq")
            wsb = perpool.tile([P, n_taps, 2, M], mybir.dt.float8e4, name="wsb")
            wz = perpool.tile([P, 2, M], mybir.dt.float8e4, name="wz")
            wmr = perpool.tile([P, 2, 2 * w], mybir.dt.float8e4, name="wmr")
            nc.gpsimd.memset(wz[:], 0.0)
            nc.gpsimd.memset(wmr[:], 0.0)

            def emit_input_rows(r0, nrows):
                nc.sync.dma_start(
                    out=xq[:, r0 + 1 : r0 + nrows + 1, :, :],
                    in_=x_dram[:, r0 : r0 + nrows, :, :],
                )

            first_piece = 4
            emit_input_rows(0, first_piece)
            nc.sync.dma_start(out=wsb[:], in_=w_dram[:])
            # top/bottom zero pad rows (full 512B line-aligned rows)
            nc.vector.memset(xq[:, 0, :, :], 0.0)
            nc.vector.memset(xq[:, hp - 1, :, :], 0.0)

            def emit_body(first):
                next_row = [first_piece if first else 0]

                def load_until(row_needed):
                    while next_row[0] < min(row_needed, h):
                        r0 = next_row[0]
                        nrows = min(4 if r0 < 12 else chunk_rows, h - r0)
                        emit_input_rows(r0, nrows)
                        next_row[0] += nrows

                if warmup_mms:
                    wpt = ppool.tile(
                        [M, 512], mybir.dt.float32, name="wpt", tag="pt",
                        padded_shape=[M, 512],
                    )
                    for _ in range(warmup_mms):
                        nc.tensor.matmul(
                            wpt[:], wz[:], wmr[:], start=True, stop=True,
                            perf_mode=mybir.MatmulPerfMode.DoubleRow,
                        )

                n_strips = h // 2
                ot = None
                for s in range(n_strips):
                    load_until(min(2 * s + 4 + chunk_rows, h))
                    pts = ppool.tile(
                        [M, 2, w], mybir.dt.float32, name="pt", tag="pt",
                    )
                    # complete each row's chain before starting the next:
                    # interleaved DoubleRow accumulation chains on one
                    # PSUM bank corrupt the first chain on real HW.
                    # Taps ordered so a full-width dx=1 tap opens the
                    # chain; dx=0/2 taps accumulate ragged 255-col ranges.
                    order = [(dy, dx) for dy in range(3) for dx in (1, 0, 2)]
                    for j in range(2):
                        for ti, (dy, dx) in enumerate(order):
                            row = 2 * s + j + dy
                            if dx == 0:
                                rhs = xq[:, row, :, 0 : w - 1]
                                dst = pts[:, j, 1:w]
                            elif dx == 1:
                                rhs = xq[:, row, :, 0:w]
                                dst = pts[:, j, 0:w]
                            else:
                                rhs = xq[:, row, :, 1:w]
                                dst = pts[:, j, 0 : w - 1]
                            nc.tensor.matmul(
                                dst,
                                wsb[:, dy * 3 + dx, :, :],
                                rhs,
                                start=(ti == 0),
                                stop=(ti == n_taps - 1),
                                perf_mode=mybir.MatmulPerfMode.DoubleRow,
                            )
                    solo = s >= n_strips - 2
                    if s % 2 == 0 or solo:
                        ot = opool.tile(
                            [M, 2 if solo else 4, w], mybir.dt.float16,
                            name="ot", tag="ot", padded_shape=[M, 4, w],
                        )
                    half = 0 if (s % 2 == 0 or solo) else 1
                    nc.scalar.mul(ot[:, 2 * half : 2 * half + 2, :],
                                  pts[:], 1.0 / div)
                    if half == 1 or solo:
                        r0 = 2 * s - (0 if solo else 2)
                        nc.sync.dma_start(
                            out=out_dram[:, r0 : r0 + ot.shape[1], :],
                            in_=ot[:],
                        )
                load_until(h)

            for _rep in range(repeats):
                emit_body(first=(_rep == 0))

    nc.compile()
    return nc


def make_weight_tensor(weight, nimg=N_IMG, cin=C_IN, cout=C_OUT):
    n_taps = 9
    wbin = np.where(weight > 0, 1.0, -1.0).astype(np.float32)
    wt = wbin.reshape(cout, cin, n_taps).transpose(1, 2, 0)
    w9 = np.zeros((nimg * cin, n_taps, nimg * cout), dtype=np.float32)
    for i in range(nimg):
        w9[i * cin : (i + 1) * cin, :, i * cout : (i + 1) * cout] = wt
    return np.repeat(w9[:, :, None, :], 2, axis=2).astype(ml_dtypes.float8_e4m3fn)


def split_fp8(x):
    hi = x.astype(ml_dtypes.float8_e4m3fn)
    lo = (x - hi.astype(np.float32)).astype(ml_dtypes.float8_e4m3fn)
    return hi, lo


def kernel(x, weight, trace=False, repeats=1, _nc_cache={}):
    x = np.asarray(x, dtype=np.float32)
    weight = np.asarray(weight, dtype=np.float32)
    per_core = x.shape[0] // N_CORES
    if repeats not in _nc_cache:
        _nc_cache[repeats] = build_conv_kernel(repeats=repeats)
    nc = _nc_cache[repeats]
    w9d = make_weight_tensor(weight)
    P = N_IMG * C_IN
    hi, lo = split_fp8(x)
    # [cores, P, H, 2, W]: planes row-interleaved
    xq = np.ascontiguousarray(np.stack(
        [hi.reshape(N_CORES, P, H, W), lo.reshape(N_CORES, P, H, W)], axis=3
    ))
    in_maps = [{"xq": xq[i], "w9d": w9d} for i in range(N_CORES)]
    try:
        res = bass_utils.run_bass_kernel_spmd(
            nc, in_maps, core_ids=list(range(N_CORES)), trace=trace
        )
    except ModuleNotFoundError:
        res = bass_utils.run_bass_kernel_spmd(
            nc, in_maps, core_ids=list(range(N_CORES)), trace=False
        )
    out = np.concatenate(
        [r["out"].astype(np.float32).reshape(per_core, C_OUT, H, W)
         for r in res.results], axis=0,
    )
    if trace:
        kernel.last_results = res
    return out


# revision 60
# speedup vs baseline: 1.3255x; 1.0074x over previous
"""fp8 DoubleRow direct-conv BConv2d on 8 TRN2 NeuronCores.

out = conv2d(x, sign(weight), 'same') / sqrt(288), x (32,32,256,256) f32.

- Data-parallel over batch (4 images x 32 cin = 128 partitions/core).
- Host splits x into fp8_e4m3 planes: hi = fp8(x), lo = fp8(x - hi);
  conv is linear so conv(x) ~ conv(hi) + conv(lo) (~1e-3 rel err).
- The planes are row-interleaved in SBUF ([P, hp, 2, 320]) so a
  DoubleRow matmul's two K-tiles are the (hi, lo) rows at stride 320
  (16-bit ISA AP step limit is +-32767).  Each 3x3 tap x output row is
  ONE DoubleRow matmul: K=256 at 0.5 PE cycles/col -> 18 matmuls of
  N=256 per 2-row strip = 0.96us, vs 1.28us for fp16 F(2,3) Winograd.
- No input transforms: DVE/Pool idle; Act drains PSUM with the 1/17
  scale.  Row pitch 320B = 10 SBUF lines, data at col 32: pad columns
  (31 / 288) share no 32B line with the DMA-written data.
"""

import numpy as np
import ml_dtypes

import concourse.mybir as mybir
import concourse.tile as tile
from concourse import bacc
from concourse import bass_utils

N_CORES = 8
N_IMG = 4
C_IN = 32
C_OUT = 32
H = 256
W = 256
DIV = float(np.sqrt(C_IN * 9))
XPITCH = 320
XCOL = 32


def build_conv_kernel(
    nimg=N_IMG, cin=C_IN, cout=C_OUT, h=H, w=W,
    chunk_rows=16, div=DIV, repeats=1, warmup_mms=6,
):
    P = nimg * cin
    M = nimg * cout
    hp = h + 2
    n_taps = 9

    nc = bacc.Bacc(
        "TRN2", target_bir_lowering=False, debug=False, num_devices=N_CORES
    )
    x_dram = nc.dram_tensor(
        "xq", [P, h, 2, w], mybir.dt.float8e4, kind="ExternalInput"
    )
    w_dram = nc.dram_tensor(
        "w9d", [P, n_taps, 2, M], mybir.dt.float8e4, kind="ExternalInput"
    )
    out_dram = nc.dram_tensor(
        "out", [M, h, w], mybir.dt.float16, kind="ExternalOutput"
    )

    with tile.TileContext(nc) as tc:
        with (
            tc.tile_pool(name="persist", bufs=1) as perpool,
            tc.tile_pool(name="ostage", bufs=4) as opool,
            tc.tile_pool(name="psum", bufs=8, space="PSUM") as ppool,
        ):
            # hi|lo packed per row: exactly 512B line-aligned rows so the
            # input DMA moves >=512B runs (1x rate, no sub-512B penalty).
            # No pad columns: the dx edge taps run ragged (the dx=1 tap
            # starts each PSUM chain full-width, dx=0/2 accumulate into
            # shifted 255-col sub-ranges whose edge contribution is zero
            # by 'same' padding anyway).
            xq = perpool.tile([P, hp, 2, w], mybir.dt.float8e4, name="xq")
            wsb = perpool.tile([P, n_taps, 2, M], mybir.dt.float8e4, name="wsb")
            wz = perpool.tile([P, 2, M], mybir.dt.float8e4, name="wz")
            wmr = perpool.tile([P, 2, 2 * w], mybir.dt.float8e4, name="wmr")
            nc.gpsimd.memset(wz[:], 0.0)
            nc.gpsimd.memset(wmr[:], 0.0)

            def emit_input_rows(r0, nrows):
                nc.sync.dma_start(
                    out=xq[:, r0 + 1 : r0 + nrows + 1, :, :],
                    in_=x_dram[:, r0 : r0 + nrows, :, :],
                )

            first_piece = 4
            emit_input_rows(0, first_piece)
            nc.sync.dma_start(out=wsb[:], in_=w_dram[:])
            # top/bottom zero pad rows (full 512B line-aligned rows)
            nc.vector.memset(xq[:, 0, :, :], 0.0)
            nc.vector.memset(xq[:, hp - 1, :, :], 0.0)

            def emit_body(first):
                next_row = [first_piece if first else 0]

                def load_until(row_needed):
                    while next_row[0] < min(row_needed, h):
                        r0 = next_row[0]
                        nrows = min(4 if r0 < 12 else chunk_rows, h - r0)
                        emit_input_rows(r0, nrows)
                        next_row[0] += nrows

                if warmup_mms:
                    wpt = ppool.tile(
                        [M, 512], mybir.dt.float32, name="wpt", tag="pt",
                        padded_shape=[M, 512],
                    )
                    for _ in range(warmup_mms):
                        nc.tensor.matmul(
                            wpt[:], wz[:], wmr[:], start=True, stop=True,
                            perf_mode=mybir.MatmulPerfMode.DoubleRow,
                        )

                n_strips = h // 2
                ot = None
                for s in range(n_strips):
                    load_until(min(2 * s + 4 + chunk_rows, h))
                    pts = ppool.tile(
                        [M, 2, w], mybir.dt.float32, name="pt", tag="pt",
                    )
                    # complete each row's chain before starting the next:
                    # interleaved DoubleRow accumulation chains on one
                    # PSUM bank corrupt the first chain on real HW.
                    # Taps ordered so a full-width dx=1 tap opens the
                    # chain; dx=0/2 taps accumulate ragged 255-col ranges.
                    order = [(dy, dx) for dy in range(3) for dx in (1, 0, 2)]
                    for j in range(2):
                        for ti, (dy, dx) in enumerate(order):
                            row = 2 * s + j + dy
                            if dx == 0:
                                rhs = xq[:, row, :, 0 : w - 1]
                                dst = pts[:, j, 1:w]
                            elif dx == 1:
                                rhs = xq[:, row, :, 0:w]
                                dst = pts[:, j, 0:w]
                            else:
                                rhs = xq[:, row, :, 1:w]
                                dst = pts[:, j, 0 : w - 1]
                            nc.tensor.matmul(
                                dst,
                                wsb[:, dy * 3 + dx, :, :],
                                rhs,
                                start=(ti == 0),
                                stop=(ti == n_taps - 1),
                                perf_mode=mybir.MatmulPerfMode.DoubleRow,
                            )
                    solo = s >= n_strips - 2
                    if s % 2 == 0 or solo:
                        ot = opool.tile(
                            [M, 2 if solo else 4, w], mybir.dt.float16,
                            name="ot", tag="ot", padded_shape=[M, 4, w],
                        )
                    half = 0 if (s % 2 == 0 or solo) else 1
                    nc.scalar.mul(ot[:, 2 * half : 2 * half + 2, :],
                                  pts[:], 1.0 / div)
                    if half == 1 or solo:
                        r0 = 2 * s - (0 if solo else 2)
                        nc.sync.dma_start(
                            out=out_dram[:, r0 : r0 + ot.shape[1], :],
                            in_=ot[:],
                        )
                load_until(h)

            for _rep in range(repeats):
                emit_body(first=(_rep == 0))

    nc.compile()
    return nc


def make_weight_tensor(weight, nimg=N_IMG, cin=C_IN, cout=C_OUT):
    n_taps = 9
    wbin = np.where(weight > 0, 1.0, -1.0).astype(np.float32)
    wt = wbin.reshape(cout, cin, n_taps).transpose(1, 2, 0)
    w9 = np.zeros((nimg * cin, n_taps, nimg * cout), dtype=np.float32)
    for i in range(nimg):
        w9[i * cin : (i + 1) * cin, :, i * cout : (i + 1) * cout] = wt
    return np.repeat(w9[:, :, None, :], 2, axis=2).astype(ml_dtypes.float8_e4m3fn)


def split_fp8(x):
    hi = x.astype(ml_dtypes.float8_e4m3fn)
    lo = (x - hi.astype(np.float32)).astype(ml_dtypes.float8_e4m3fn)
    return hi, lo


def kernel(x, weight, trace=False, repeats=1, _nc_cache={}):
    x = np.asarray(x, dtype=np.float32)
    weight = np.asarray(weight, dtype=np.float32)
    per_core = x.shape[0] // N_CORES
    if repeats not in _nc_cache:
        _nc_cache[repeats] = build_conv_kernel(repeats=repeats)
    nc = _nc_cache[repeats]
    w9d = make_weight_tensor(weight)
    P = N_IMG * C_IN
    hi, lo = split_fp8(x)
    # [cores, P, H, 2, W]: planes row-interleaved
    xq = np.ascontiguousarray(np.stack(
        [hi.reshape(N_CORES, P, H, W), lo.reshape(N_CORES, P, H, W)], axis=3
    ))
    in_maps = [{"xq": xq[i], "w9d": w9d} for i in range(N_CORES)]
    try:
        res = bass_utils.run_bass_kernel_spmd(
            nc, in_maps, core_ids=list(range(N_CORES)), trace=trace
        )
    except ModuleNotFoundError:
        res = bass_utils.run_bass_kernel_spmd(
            nc, in_maps, core_ids=list(range(N_CORES)), trace=False
        )
    out = np.concatenate(
        [r["out"].astype(np.float32).reshape(per_core, C_OUT, H, W)
         for r in res.results], axis=0,
    )
    if trace:
        kernel.last_results = res
    return out
